# revision 37
# baseline (speedup 1.0000x reference)
"""BertLayer forward on 8 Trainium2 NeuronCores — minimal-upload edition.

The graded time for this problem is dominated by host->device streaming of
the input tensors, not by compute (the compute schedule is ~0.5 ms while the
baseline's 8x-replicated fp32 inputs cost ~99 ms to stream).  So inputs are
sharded on the host and reassembled on device:

  - all four weight matrices ship as bf16 row-shards [16, X] (1/8 each) and
    are AllGather'd across the 8 cores into full [128, X] blockified DRAM
    copies (wire cost ~15us + size/BW each, overlapped with compute).
  - the hidden state ships as each core's own 512-token channel-major slice
    (bf16) and is AllGather'd within each batch's 4-core group to give every
    core its batch's full [1024, 2048] hidden (as 4 token blocks).
  - per-core upload drops 50 MB -> ~3.6 MB (weights 2.5, hidden 1, misc).

Compute is the proven token-parallel schedule: 512 tokens/core, each core
recomputes its batch's full QKV (bf16 weights x bf16 hidden, fp32 PSUM), so
attention needs no communication.  Scores fold the attention mask in as a
65th contraction row; softmax denominators fall out of a 65th output row of
the probs @ v matmul; LayerNorm reductions run on the PE via ones-vector
matmuls.  FFN weights are converted bf16->fp32r per-slab under the matmuls.
"""
import numpy as np
from contextlib import ExitStack

B, S, D = 2, 2048, 1024
H, DH = 16, 64
DFF = 4096
EPS = 1e-5
NCORES = 8
TOK = (B * S) // NCORES          # 512 tokens owned per core
CPB = NCORES // B                # 4 cores per batch
CH_T = D // 128                  # 8 channel tiles
DFF_T = DFF // 128               # 32 dff tiles
T_T = S // 128                   # 16 key-token tiles
WROWS = 128 // NCORES            # 16 weight rows uploaded per core

_CACHE = {}


def _build(repeat=1):
    # repeat>1 wraps phases 1-5 in a hardware loop (collectives stay outside)
    # -- bench-only knob to measure real per-iteration compute time.
    import concourse.bass as bass
    import concourse.tile as tile
    from concourse import bacc, mybir
    from concourse.masks import make_identity

    F32 = mybir.dt.float32
    F32R = mybir.dt.float32r
    BF16 = mybir.dt.bfloat16
    AF = mybir.ActivationFunctionType
    OP = mybir.AluOpType

    nc = bacc.Bacc("TRN2", target_bir_lowering=False, debug=False,
                   num_devices=NCORES)

    # ---- sharded inputs (per-core distinct) ----
    # wq+biases ride the first AllGather (they alone gate phase 1);
    # wso+wi+wo ride the second (first needed by phase 3).  Bias block
    # layout (88 cols, in _cols layout each):
    #   qb 0:8 | sob 8:16 | ib 16:48 | ob 48:56 | l1g 56:64 | l1b 64:72
    #   | l2g 72:80 | l2b 80:88
    NB = 88
    QSO = CH_T * D + NB
    IO = CH_T * D + DFF_T * D + CH_T * DFF
    h_own = nc.dram_tensor("h_own", [D, TOK], BF16, kind="ExternalInput").ap()
    wqso_in = nc.dram_tensor("wqso_in", [WROWS, QSO], BF16, kind="ExternalInput").ap()
    wio_in = nc.dram_tensor("wio_in", [WROWS, IO], BF16, kind="ExternalInput").ap()
    mask8 = nc.dram_tensor("mask8", [1, S], F32R, kind="ExternalInput").ap()
    out = nc.dram_tensor("out", [TOK, D], BF16, kind="ExternalOutput").ap()

    GROUPS = [(i * 2, 2) for i in range(8)]
    GW = 2
    AG8 = [list(range(NCORES))]
    AG4 = [[0, 1, 2, 3], [4, 5, 6, 7]]

    with tile.TileContext(nc) as tc, ExitStack() as root:
        # ---------------- Phase 0: reassemble sharded inputs --------------
        dram = root.enter_context(tc.tile_pool(name="dram", bufs=1, space="DRAM"))
        wqso_b = dram.tile([WROWS, QSO], BF16)
        h_b = dram.tile([D, TOK], BF16)
        wio_b = dram.tile([WROWS, IO], BF16)
        wqso_full = dram.tile([128, QSO], BF16, addr_space="Shared")
        h4_full = dram.tile([CPB * D, TOK], BF16)
        wio_full = dram.tile([128, IO], BF16, addr_space="Shared")
        nc.sync.dma_start(wqso_b[:], wqso_in[:])
        nc.sync.dma_start(h_b[:], h_own[:])
        nc.sync.dma_start(wio_b[:], wio_in[:])
        for src, dst, rg in ((wqso_b, wqso_full, AG8), (h_b, h4_full, AG4),
                             (wio_b, wio_full, AG8)):
            nc.gpsimd.collective_compute(
                "AllGather", mybir.AluOpType.bypass, replica_groups=rg,
                ins=[src.opt()], outs=[dst.opt()])
        wq_full = wqso_full[:, 0:CH_T * D]
        bias_full = wqso_full[:, CH_T * D:QSO]
        wso_full = wio_full[:, 0:CH_T * D]
        wi_full = wio_full[:, CH_T * D:CH_T * D + DFF_T * D]
        wo_full = wio_full[:, CH_T * D + DFF_T * D:IO]

        const = root.enter_context(tc.tile_pool(name="const", bufs=1))
        ones2_f = const.tile([128, 2], F32, tag="ones2f")
        nc.vector.memset(ones2_f[:], 1.0)
        ones_col = const.tile([128, 1], F32R, tag="onescol")
        nc.vector.tensor_copy(ones_col[:], ones2_f[:, 0:1])
        ones_row = const.tile([1, TOK], F32, tag="onesrowf")
        nc.vector.memset(ones_row[:], 1.0)
        ident_f = const.tile([128, 128], F32, tag="identf")
        make_identity(nc, ident_f[:])
        ident_r = const.tile([128, 128], F32R, tag="identr")
        nc.vector.tensor_copy(ident_r[:], ident_f[:])

        bias_p = root.enter_context(tc.tile_pool(name="bias", bufs=1))
        bias_bf = bias_p.tile([128, NB], BF16, tag="biasbf")
        nc.sync.dma_start(bias_bf[:], bias_full)
        bias_all = bias_p.tile([128, NB], F32, tag="biasall")
        nc.vector.tensor_copy(bias_all[:], bias_bf[:])
        qb_s = bias_all[:, 0:8]
        sob_s = bias_all[:, 8:16]
        ib_s = bias_all[:, 16:48]
        ob_s = bias_all[:, 48:56]
        l1g_s = bias_all[:, 56:64]
        l1b_s = bias_all[:, 64:72]
        l2g_s = bias_all[:, 72:80]
        l2b_s = bias_all[:, 80:88]

        # DVE scratch shared by LN phases
        scr = root.enter_context(tc.tile_pool(name="scratch", bufs=2))

        loop_cm = tc.For_i(0, repeat, 1) if repeat > 1 else None
        if loop_cm is not None:
            loop_cm.__enter__()

        # long-lived activation tensors, opened in LIFO-compatible order
        xln_scope = ExitStack()
        xlnp = xln_scope.enter_context(tc.tile_pool(name="xln", bufs=1))
        xln = xlnp.tile([128, CH_T * TOK], BF16, tag="xln")

        attn_scope = ExitStack()
        attnp = attn_scope.enter_context(tc.tile_pool(name="attn", bufs=1))
        attnT = attnp.tile([128, CH_T * TOK], BF16, tag="attnT")

        qkv_scope = ExitStack()
        qkvp = qkv_scope.enter_context(tc.tile_pool(name="qkvT", bufs=1))
        qkvT = qkvp.tile([128, CH_T * S], F32R, tag="qkvT")
        qkvOwn = qkvp.tile([128, CH_T * TOK], F32R, tag="qkvOwn")

        # own hidden slice: bf16 for the QKV matmul (dies with phase 2),
        # fp32 for the LN1 residual (dies with phase 3)
        hown_bf = qkvp.tile([128, CH_T * TOK], BF16, tag="hownbf")
        for k in range(CH_T):
            nc.sync.dma_start(hown_bf[:, k * TOK:(k + 1) * TOK],
                              h_own[k * 128:(k + 1) * 128, :])
        hown_f = attnp.tile([128, CH_T * TOK], F32, tag="hownf")
        nc.vector.tensor_copy(hown_f[:], hown_bf[:])

        # ---------------- Phase 1: qkvT = wq @ h  (full batch) ------------
        with tc.tile_pool(name="wq_p", bufs=1) as wq_p, \
             tc.tile_pool(name="ht_p", bufs=1) as ht_p, \
             tc.tile_pool(name="ps_qkv", bufs=3, space="PSUM") as ps_qkv:
            wq_s = wq_p.tile([128, CH_T * D], BF16, tag="wq")
            nc.sync.dma_start(wq_s[:], wq_full[:])
            # own block first (needs only wq + local h; overlaps the h AG)
            for m in range(CH_T):
                ps = ps_qkv.tile([128, TOK], F32, tag="pso")
                for k in range(CH_T):
                    nc.tensor.matmul(
                        ps[:], wq_s[:, m * D + k * 128:m * D + k * 128 + 128],
                        hown_bf[:, k * TOK:(k + 1) * TOK],
                        start=(k == 0), stop=(k == CH_T - 1))
                nc.vector.tensor_scalar_add(
                    qkvOwn[:, m * TOK:(m + 1) * TOK], ps[:], qb_s[:, m:m + 1])
            # full batch from the gathered hidden (token block n = AG rank n).
            # m-outer so each qkvT head-slab completes early and phase 2 can
            # start its (ACT-bound) softmax stream while later slabs compute.
            # n innermost: one wq stationary block serves all 4 token blocks
            # back-to-back (4 PSUM banks accumulate in parallel).
            ht = []
            for n in range(CPB):
                for k in range(CH_T):
                    t = ht_p.tile([128, TOK], BF16, tag=f"ht{n}_{k}",
                                  name=f"ht{n}_{k}")
                    nc.sync.dma_start(
                        t[:], h4_full[n * D + k * 128:n * D + (k + 1) * 128, :])
                    ht.append(t)
            for m in range(CH_T):
                psn = [ps_qkv.tile([128, TOK], F32, tag=f"psn{n}",
                                   name=f"psn{n}", bufs=1) for n in range(CPB)]
                for k in range(CH_T):
                    for n in range(CPB):
                        nc.tensor.matmul(
                            psn[n][:],
                            wq_s[:, m * D + k * 128:m * D + k * 128 + 128],
                            ht[n * CH_T + k][:], start=(k == 0),
                            stop=(k == CH_T - 1))
                for n in range(CPB):
                    nc.vector.tensor_scalar_add(
                        qkvT[:, m * S + n * TOK:m * S + (n + 1) * TOK],
                        psn[n][:], qb_s[:, m:m + 1])

        # ---------------- Phase 2: attention ------------------------------
        with tc.tile_pool(name="vA_p", bufs=2) as vA_p, \
             tc.tile_pool(name="ktaug_p", bufs=3) as kt_p, \
             tc.tile_pool(name="qtaug_p", bufs=3) as qt_p, \
             tc.tile_pool(name="ps_tr", bufs=2, space="PSUM") as ps_tr, \
             tc.tile_pool(name="ps_sc", bufs=2, space="PSUM") as ps_sc, \
             tc.tile_pool(name="ps_at", bufs=2, space="PSUM") as ps_at, \
             tc.tile_pool(name="probs_p", bufs=4) as probs_p, \
             tc.tile_pool(name="rec_p", bufs=2) as rec_p:
            for m in range(CH_T):
                # v for heads 2m, 2m+1: transpose qkvT chunk to token-major,
                # interleave a ones column per head for the softmax denom.
                vA = vA_p.tile([128, T_T * 130], F32R, tag="vA")
                for i in range(T_T):
                    pt = ps_tr.tile([128, 128], F32R, tag="pt")
                    nc.tensor.transpose(
                        pt[:], qkvT[:, m * S + i * 128:m * S + (i + 1) * 128],
                        ident_r[:])
                    dst = vA[:, i * 130:(i + 1) * 130].rearrange(
                        "p (g c) -> p g c", c=65)[:, :, 0:64]
                    src = pt[:].rearrange("p (g c) -> p g c", g=2)
                    nc.vector.tensor_copy(dst, src)
                    ones_dst = vA[:, i * 130:(i + 1) * 130].rearrange(
                        "p (g c) -> p g c", c=65)[:, :, 64:65]
                    nc.vector.tensor_copy(
                        ones_dst, ones2_f[:].rearrange("p (g c) -> p g c", c=1))
                for sub in range(2):
                    h0 = sub * 64
                    ktaug = kt_p.tile([65, S], F32R, tag="ktaug")
                    nc.vector.tensor_copy(
                        ktaug[0:64, :], qkvT[h0:h0 + 64, m * S:(m + 1) * S])
                    nc.sync.dma_start(ktaug[64:65, :], mask8[:])
                    qtaug = qt_p.tile([65, TOK], F32R, tag="qtaug")
                    nc.vector.tensor_copy(
                        qtaug[0:64, :], qkvOwn[h0:h0 + 64, m * TOK:(m + 1) * TOK])
                    nc.vector.tensor_copy(qtaug[64:65, :], ones_row[:])

                    pat = ps_at.tile([65, TOK], F32, tag="pat")
                    for g0, glen in GROUPS:
                        psc = ps_sc.tile([128, GW * 512], F32, tag="psc")
                        for j in range(glen):
                            i = g0 + j
                            nc.tensor.matmul(
                                psc[:, j * 512:(j + 1) * 512],
                                ktaug[:, i * 128:(i + 1) * 128], qtaug[:],
                                start=True, stop=True)
                        probs = probs_p.tile([128, GW * 512], F32R, tag="probs")
                        nc.scalar.activation(
                            probs[:, 0:glen * 512], psc[:, 0:glen * 512],
                            AF.Exp, scale=float(1.0 / np.sqrt(DH)))
                        for j in range(glen):
                            i = g0 + j
                            nc.tensor.matmul(
                                pat[:],
                                vA[:, i * 130 + sub * 65:i * 130 + sub * 65 + 65],
                                probs[:, j * 512:(j + 1) * 512],
                                start=(i == 0), stop=(i == T_T - 1))
                    rec = rec_p.tile([1, TOK], F32, tag="rec")
                    nc.vector.reciprocal(rec[:], pat[64:65, :])
                    recb = rec_p.tile([64, TOK], F32, tag="recb")
                    nc.gpsimd.partition_broadcast(recb[:], rec[:])
                    nc.vector.tensor_mul(
                        attnT[h0:h0 + 64, m * TOK:(m + 1) * TOK],
                        pat[0:64, :], recb[:])
        qkv_scope.close()

        # ---------------- Phase 3: self-output + LN1 ----------------------
        with tc.tile_pool(name="wso_p", bufs=3) as wso_p, \
             tc.tile_pool(name="x_p", bufs=1) as x_p, \
             tc.tile_pool(name="ps_so", bufs=3, space="PSUM") as ps_so, \
             tc.tile_pool(name="ps_sum", bufs=1, space="PSUM") as ps_sum, \
             tc.tile_pool(name="ln_small", bufs=1) as lnp, \
             tc.tile_pool(name="lnb_p", bufs=1) as lnb_p:
            x_sb = x_p.tile([128, CH_T * TOK], F32R, tag="x")
            pss = ps_sum.tile([1, TOK], F32, tag="s")
            psq = ps_sum.tile([1, TOK], F32, tag="q")
            for m in range(CH_T):
                wsom_b = wso_p.tile([128, D], BF16, tag="wsomb")
                nc.sync.dma_start(wsom_b[:], wso_full[:, m * D:(m + 1) * D])
                ps = ps_so.tile([128, TOK], F32, tag="ps")
                for k in range(CH_T):
                    nc.tensor.matmul(
                        ps[:], wsom_b[:, k * 128:(k + 1) * 128],
                        attnT[:, k * TOK:(k + 1) * TOK],
                        start=(k == 0), stop=(k == CH_T - 1))
                xs = x_sb[:, m * TOK:(m + 1) * TOK]
                nc.vector.scalar_tensor_tensor(
                    xs, ps[:], sob_s[:, m:m + 1],
                    hown_f[:, m * TOK:(m + 1) * TOK], OP.add, OP.add)
                sq = scr.tile([128, TOK], F32R, tag="sq")
                nc.vector.tensor_mul(sq[:], xs, xs)
                nc.tensor.matmul(pss[:], ones_col[:], xs,
                                 start=(m == 0), stop=(m == CH_T - 1))
                nc.tensor.matmul(psq[:], ones_col[:], sq[:],
                                 start=(m == 0), stop=(m == CH_T - 1))

            mu = lnp.tile([1, TOK], F32, tag="mu1")
            ex2 = lnp.tile([1, TOK], F32, tag="ex21")
            nc.scalar.mul(mu[:], pss[:], 1.0 / D)
            nc.scalar.mul(ex2[:], psq[:], 1.0 / D)
            sqmu = lnp.tile([1, TOK], F32, tag="sqmu1")
            nc.vector.tensor_mul(sqmu[:], mu[:], mu[:])
            vare = lnp.tile([1, TOK], F32, tag="vare1")
            nc.vector.scalar_tensor_tensor(vare[:], ex2[:], EPS, sqmu[:],
                                           OP.add, OP.subtract)
            rcp = lnp.tile([1, TOK], F32, tag="rcp1")
            nc.vector.reciprocal(rcp[:], vare[:])
            rstd = lnp.tile([1, TOK], F32, tag="rstd1")
            nc.scalar.sqrt(rstd[:], rcp[:])
            rstd_b = lnb_p.tile([128, TOK], F32, tag="rstdb1")
            mu_b = lnb_p.tile([128, TOK], F32, tag="mub1")
            nc.gpsimd.partition_broadcast(rstd_b[:], rstd[:])
            nc.gpsimd.partition_broadcast(mu_b[:], mu[:])
            for m in range(CH_T):
                xs = x_sb[:, m * TOK:(m + 1) * TOK]
                d = scr.tile([128, TOK], F32, tag="d")
                nc.vector.tensor_sub(d[:], xs, mu_b[:])
                e = scr.tile([128, TOK], F32, tag="e")
                nc.vector.scalar_tensor_tensor(
                    e[:], d[:], l1g_s[:, m:m + 1], rstd_b[:], OP.mult, OP.mult)
                nc.vector.tensor_scalar_add(
                    xln[:, m * TOK:(m + 1) * TOK], e[:], l1b_s[:, m:m + 1])
        attn_scope.close()

        # ---------------- Phase 4: FFN1 + GELU + partial FFN2 -------------
        # g is bf16 so FFN2 runs bf16 x bf16 with no weight conversion.  The
        # first NOV output slabs of FFN2 accumulate in persistent PSUM banks
        # as each g slab appears, shortening the serial FFN2 tail.
        NOV = 4
        g_scope = ExitStack()
        gp = g_scope.enter_context(tc.tile_pool(name="g_p", bufs=1))
        g_sb = gp.tile([128, DFF_T * TOK], BF16, tag="g")
        woh_p = g_scope.enter_context(tc.tile_pool(name="woh", bufs=1))
        ps_z = g_scope.enter_context(tc.tile_pool(name="ps_z", bufs=1,
                                                  space="PSUM"))
        woh, zps = [], []
        for mz in range(NOV):
            w = woh_p.tile([128, DFF], BF16, tag=f"woh{mz}", name=f"woh{mz}")
            nc.sync.dma_start(w[:], wo_full[:, mz * DFF:(mz + 1) * DFF])
            woh.append(w)
            zps.append(ps_z.tile([128, TOK], F32, tag=f"z{mz}", name=f"z{mz}"))
        with tc.tile_pool(name="wi_p", bufs=6) as wi_p, \
             tc.tile_pool(name="ps_f1", bufs=3, space="PSUM") as ps_f1:
            for m in range(DFF_T):
                wim_b = wi_p.tile([128, D], BF16, tag="wimb")
                nc.sync.dma_start(wim_b[:], wi_full[:, m * D:(m + 1) * D])
                ps = ps_f1.tile([128, TOK], F32, tag="ps")
                for k in range(CH_T):
                    nc.tensor.matmul(
                        ps[:], wim_b[:, k * 128:(k + 1) * 128],
                        xln[:, k * TOK:(k + 1) * TOK],
                        start=(k == 0), stop=(k == CH_T - 1))
                gs = g_sb[:, m * TOK:(m + 1) * TOK]
                nc.scalar.activation(gs, ps[:], AF.Gelu, bias=ib_s[:, m:m + 1])
                for mz in range(NOV):
                    nc.tensor.matmul(
                        zps[mz][:], woh[mz][:, m * 128:(m + 1) * 128], gs,
                        start=(m == 0), stop=(m == DFF_T - 1))

        # ---------------- Phase 5: FFN2 + LN2 + transpose out -------------
        with tc.tile_pool(name="wo_p", bufs=2) as wo_p, \
             tc.tile_pool(name="ps_f2", bufs=2, space="PSUM") as ps_f2, \
             tc.tile_pool(name="z_p", bufs=1) as z_p, \
             tc.tile_pool(name="ps_sum2", bufs=1, space="PSUM") as ps_sum2, \
             tc.tile_pool(name="ln2_small", bufs=1) as ln2p, \
             tc.tile_pool(name="ln2b_p", bufs=1) as ln2b_p, \
             tc.tile_pool(name="y_p", bufs=2) as y_p, \
             tc.tile_pool(name="stage_p", bufs=1) as stage_p:
            z_sb = z_p.tile([128, CH_T * TOK], F32R, tag="z")
            pss2 = ps_sum2.tile([1, TOK], F32, tag="s")
            psq2 = ps_sum2.tile([1, TOK], F32, tag="q")
            for m in range(CH_T):
                if m < NOV:
                    ps = zps[m]
                else:
                    wom_b = wo_p.tile([128, DFF], BF16, tag="womb")
                    nc.sync.dma_start(wom_b[:],
                                      wo_full[:, m * DFF:(m + 1) * DFF])
                    ps = ps_f2.tile([128, TOK], F32, tag="ps")
                    for k in range(DFF_T):
                        nc.tensor.matmul(
                            ps[:], wom_b[:, k * 128:(k + 1) * 128],
                            g_sb[:, k * TOK:(k + 1) * TOK],
                            start=(k == 0), stop=(k == DFF_T - 1))
                zs = z_sb[:, m * TOK:(m + 1) * TOK]
                nc.vector.scalar_tensor_tensor(
                    zs, ps[:], ob_s[:, m:m + 1],
                    xln[:, m * TOK:(m + 1) * TOK], OP.add, OP.add)
                sq = scr.tile([128, TOK], F32R, tag="sq")
                nc.vector.tensor_mul(sq[:], zs, zs)
                nc.tensor.matmul(pss2[:], ones_col[:], zs,
                                 start=(m == 0), stop=(m == CH_T - 1))
                nc.tensor.matmul(psq2[:], ones_col[:], sq[:],
                                 start=(m == 0), stop=(m == CH_T - 1))

            mu2 = ln2p.tile([1, TOK], F32, tag="mu2")
            ex22 = ln2p.tile([1, TOK], F32, tag="ex22")
            nc.scalar.mul(mu2[:], pss2[:], 1.0 / D)
            nc.scalar.mul(ex22[:], psq2[:], 1.0 / D)
            sqmu2 = ln2p.tile([1, TOK], F32, tag="sqmu2")
            nc.vector.tensor_mul(sqmu2[:], mu2[:], mu2[:])
            vare2 = ln2p.tile([1, TOK], F32, tag="vare2")
            nc.vector.scalar_tensor_tensor(vare2[:], ex22[:], EPS, sqmu2[:],
                                           OP.add, OP.subtract)
            rcp2 = ln2p.tile([1, TOK], F32, tag="rcp2")
            nc.vector.reciprocal(rcp2[:], vare2[:])
            rstd2 = ln2p.tile([1, TOK], F32, tag="rstd2")
            nc.scalar.sqrt(rstd2[:], rcp2[:])
            rstd2_b = ln2b_p.tile([128, TOK], F32, tag="rstdb2")
            mu2_b = ln2b_p.tile([128, TOK], F32, tag="mub2")
            nc.gpsimd.partition_broadcast(rstd2_b[:], rstd2[:])
            nc.gpsimd.partition_broadcast(mu2_b[:], mu2[:])

            stage = stage_p.tile([128, (TOK // 128) * D], BF16, tag="stage")
            for m in range(CH_T):
                zs = z_sb[:, m * TOK:(m + 1) * TOK]
                d = scr.tile([128, TOK], F32, tag="d")
                nc.vector.tensor_sub(d[:], zs, mu2_b[:])
                e = scr.tile([128, TOK], F32, tag="e")
                nc.vector.scalar_tensor_tensor(
                    e[:], d[:], l2g_s[:, m:m + 1], rstd2_b[:], OP.mult, OP.mult)
                y_m = y_p.tile([128, TOK], F32, tag="y")
                nc.vector.tensor_scalar_add(y_m[:], e[:], l2b_s[:, m:m + 1])
                for j in range(TOK // 128):
                    pt = ps_f2.tile([128, 128], F32, tag="ps")
                    nc.tensor.transpose(
                        pt[:], y_m[:, j * 128:(j + 1) * 128], ident_f[:])
                    nc.scalar.copy(
                        stage[:, j * D + m * 128:j * D + (m + 1) * 128], pt[:])
            for j in range(TOK // 128):
                nc.sync.dma_start(out[j * 128:(j + 1) * 128, :],
                                  stage[:, j * D:(j + 1) * D])
        g_scope.close()
        xln_scope.close()
        if loop_cm is not None:
            loop_cm.__exit__(None, None, None)
    nc.finalize()
    return nc


def _blockify(wt, kt, mt):
    # wt: [kt*128, mt*128] (already W.T). Block (m, k) lands at columns
    # [m*kt*128 + k*128, ...+128) so a per-m slab is one contiguous DMA.
    return np.ascontiguousarray(
        wt.reshape(kt, 128, mt, 128).transpose(1, 2, 0, 3).reshape(128, -1))


def _cols(bias, nt):
    return np.ascontiguousarray(np.asarray(bias, np.float32).reshape(nt, 128).T)


def _bf16(a):
    from concourse import mybir
    return np.asarray(a, np.float32).astype(mybir.dt.np(mybir.dt.bfloat16))


def _in_maps(hidden_state, attention_mask, q_w, q_b, so_w, so_b, ln1_g, ln1_b,
             inter_w, inter_b, out_w, out_b, ln2_g, ln2_b):
    wqso_blk = _bf16(np.concatenate([
        _blockify(np.asarray(q_w, np.float32).T, CH_T, CH_T),
        _cols(q_b, CH_T), _cols(so_b, CH_T),
        _cols(inter_b, DFF_T), _cols(out_b, CH_T),
        _cols(ln1_g, CH_T), _cols(ln1_b, CH_T),
        _cols(ln2_g, CH_T), _cols(ln2_b, CH_T)], axis=1))
    wio_blk = _bf16(np.concatenate([
        _blockify(np.asarray(so_w, np.float32).T, CH_T, CH_T),
        _blockify(np.asarray(inter_w, np.float32).T, CH_T, DFF_T),
        _blockify(np.asarray(out_w, np.float32).T, DFF_T, CH_T)], axis=1))
    hidden_state = np.asarray(hidden_state, np.float32)
    attention_mask = np.asarray(attention_mask, np.float32)
    hT = [_bf16(np.ascontiguousarray(hidden_state[b].T)) for b in range(B)]
    in_maps = []
    for c in range(NCORES):
        b, r = divmod(c, CPB)
        m8 = (8.0 * attention_mask[b, 0, 0, :]).reshape(1, S)
        in_maps.append({
            "h_own": np.ascontiguousarray(hT[b][:, r * TOK:(r + 1) * TOK]),
            "wqso_in": np.ascontiguousarray(wqso_blk[c * WROWS:(c + 1) * WROWS, :]),
            "wio_in": np.ascontiguousarray(wio_blk[c * WROWS:(c + 1) * WROWS, :]),
            "mask8": np.ascontiguousarray(m8.astype(np.float32)),
        })
    return in_maps


def kernel(hidden_state, attention_mask, q_w, q_b, so_w, so_b, ln1_g, ln1_b,
           inter_w, inter_b, out_w, out_b, ln2_g, ln2_b):
    from concourse.bass_utils import run_bass_kernel_spmd

    if "nc" not in _CACHE:
        _CACHE["nc"] = _build()
    nc = _CACHE["nc"]

    in_maps = _in_maps(hidden_state, attention_mask, q_w, q_b, so_w, so_b,
                       ln1_g, ln1_b, inter_w, inter_b, out_w, out_b,
                       ln2_g, ln2_b)
    res = run_bass_kernel_spmd(nc, in_maps, list(range(NCORES)))
    full = np.empty((B, S, D), np.float32)
    for c in range(NCORES):
        b, r = divmod(c, CPB)
        full[b, r * TOK:(r + 1) * TOK, :] = res.results[c]["out"].astype(np.float32)
    return full
